# revision 25
# baseline (speedup 1.0000x reference)
"""Trainium2 Bass kernel for CustomMultiHeadSelfAttention (fused q/k LayerNorm).

Reference computation (per batch n):
    q = x @ Wq.T ; k = x @ Wk.T ; v = x @ Wv.T          (split into 16 heads of 64)
    q = LN_head(q) * gq + bq ; k = LN_head(k) * gk + bk  (LayerNorm over head_dim)
    out = causal_softmax(q @ k.T) @ v                    (per head)
    y = concat_heads(out) @ Wo.T + bo

Sharding: 8 cores = 2 batches x 4 head-groups (4 heads each).  Each core
computes its heads' attention and a partial y = out_heads @ Wo[:, cols].T;
the host sums the 4 partials per batch (fp32) and adds bo.

Device-side dataflow per core (all matmul operands bf16, PSUM accum fp32):
  - QKV projection from streamed xT chunks; LayerNorm mean is folded into
    the Q/K weights on the host (centered weights => mean(q)=0), so LN
    reduces to q * rsqrt(mean(q^2)+eps).
  - Engine split chosen so ACT only ever runs functions from ONE table set
    (natural_log_exp_and_others: Exp/Ln/Square/Identity/Copy -> a single
    table load): squares for LN stats on ACT, grouped sum on DVE,
    rstd = exp(-0.5*ln(ms+eps)) on ACT, q/k scaling merged per-tile on DVE
    (stride-0 broadcast of per-head rstd), PE transpose to [d, l] (bf16,
    1 cyc/row), *g+b fused into the ACT Identity PSUM->SBUF store.
    Transposes are software-pipelined one tile behind the projection
    matmuls so the PE never waits on the DVE/ACT consumers.
  - Attention is l-chunk-major: for each 512-wide query chunk c, S^T for
    m-chunks j=0..4c+3 is computed in j-pairs into a [P, 2, 512] PSUM
    tile, exp'd in one ACT pass (fp32->bf16, no max subtraction: LN
    bounds |score| <= 64), and accumulated V-stationary into a single
    per-head O PSUM bank: OT_aug[65, 512] += V_aug[m,65]^T @ P[m, 512]
    (V_aug has a ones column -> row 64 = softmax denominators).  The
    causal diagonal 128x128 blocks get a -1e30 additive mask via an
    identity matmul.  O rows are scaled by 1/sums (DVE reciprocal +
    GPSIMD partition-broadcast + DVE multiply) into bf16 OT_heads.
  - One O bank per head (vs 4 in a j-major sweep) leaves 2 PSUM banks for
    the output projection, which is interleaved per chunk right after the
    chunk's last head is normalized: y[t] = sum_p OT_p[:,t]^T @ WoT_p,
    PSUM->SBUF bf16 copies alternating DVE/ACT, streamed to DRAM as bf16.
"""

import ml_dtypes
import numpy as np

import concourse.bass as bass
import concourse.tile as tile
from concourse import bacc, mybir
from concourse.bass_utils import run_bass_kernel_spmd

F32 = mybir.dt.float32
BF16 = mybir.dt.bfloat16
F16 = mybir.dt.float16

P = 128
EMB = 1024
L = 2048
D = 64
HPC = 4           # heads per core
NCORES = 8
EPS = 1e-5
NEG = -60000.0  # f16-finite; scores are bounded by 64, exp(s+NEG) == 0
T = L // P        # 16 l-tiles
E = EMB // P      # 8 emb chunks
C = 4             # 512-wide query chunks
AF = mybir.ActivationFunctionType
ALU = mybir.AluOpType


def build_nc():
    nc = bacc.Bacc("TRN2", target_bir_lowering=False, debug=False, num_devices=NCORES)

    xT_d = nc.dram_tensor("xT", [EMB, L], F16, kind="ExternalInput")
    wqv_d = nc.dram_tensor("wqv", [E, P, 512], F16, kind="ExternalInput")
    wk_d = nc.dram_tensor("wk", [E, P, 256], F16, kind="ExternalInput")
    wo_d = nc.dram_tensor("wo", [2, P, EMB], F16, kind="ExternalInput")
    cst_d = nc.dram_tensor("cst", [P, 2, P], F16, kind="ExternalInput")  # ident|maskf
    gb_d = nc.dram_tensor("gb", [P, 4], F32, kind="ExternalInput")  # gq2 bq2 gk2 bk2
    y_d = nc.dram_tensor("y", [L, EMB], F16, kind="ExternalOutput")

    with tile.TileContext(nc) as tc:
        # ---- persistent pools (bottom of the SBUF stack) ----
        with (
            tc.tile_pool(name="const", bufs=1) as const_p,
            tc.tile_pool(name="vbuf", bufs=1) as vbuf_p,
            tc.tile_pool(name="qtkt", bufs=1) as qtkt_p,
        ):
            cst = const_p.tile([P, 2, P], F16, tag="cst")
            ident = cst[:, 0, :]
            maskf = cst[:, 1, :]
            gb = const_p.tile([P, 4], F32, tag="gb")
            epst = const_p.tile([P, 1], F32, tag="epst")
            # all ACT functions used below (Square/Ln/Exp/Identity/Copy) live
            # in table set 6 (natural_log_exp_and_others); pre-placing the
            # load keeps the fixpoint pass from greedy-thrashing between sets
            nc.scalar.add_instruction(mybir.InstLoadActFuncSet(
                name=nc.get_next_instruction_name(), act_func_set_id=6,
                ins=[], outs=[]))
            nc.vector.memset(epst[:], EPS)

            # V with a ones column per head: vb[t][:, h, 0:65]
            vb = []
            for t in range(T):
                v_ = vbuf_p.tile([P, HPC, 65], BF16, tag=f"vb{t}", name=f"vb{t}")
                nc.gpsimd.memset(v_[:], 1.0)  # ones col survives V copy
                vb.append(v_)

            # QT/KT: head pair p occupies rows [0:64]=head 2p, [64:128]=head 2p+1
            qt = [qtkt_p.tile([P, L], F16, tag=f"qt{p_}", name=f"qt{p_}") for p_ in range(2)]
            kt = [qtkt_p.tile([P, L], F16, tag=f"kt{p_}", name=f"kt{p_}") for p_ in range(2)]

            # ================= Phase 1: projections + LN + transpose ==========
            with (
                tc.tile_pool(name="xt", bufs=1) as xt_p,
                tc.tile_pool(name="wts", bufs=1) as wts_p,
                tc.tile_pool(name="rows", bufs=3) as rows_p,
                tc.tile_pool(name="stats", bufs=3) as stats_p,
                tc.tile_pool(name="ps_qv", bufs=3, space="PSUM") as ps_qv,
                tc.tile_pool(name="ps_k", bufs=3, space="PSUM") as ps_k,
                tc.tile_pool(name="ps_tr", bufs=2, space="PSUM") as ps_tr,
            ):
                # batched input streaming: every dma_start costs ~625ns of
                # HWDGE descriptor generation, so issue few large transfers
                # ordered so tile 0's first matmuls unblock earliest
                xtd = xT_d.rearrange("(e p) l -> e p l", p=P)
                wqvt, wkt = [], []
                xth = [[None, None] for _ in range(4)]  # [quarter][e-half]

                def load_w(h, split=False):
                    wq_ = wts_p.tile([P, 4, 512], F16, tag=f"wqv{h}", name=f"wqv{h}")
                    wk_ = wts_p.tile([P, 4, 256], F16, tag=f"wk{h}", name=f"wk{h}")
                    halves = [(0, 2), (2, 4)] if split else [(0, 4)]
                    for (a, b) in halves:
                        nc.sync.dma_start(
                            wq_[:, a:b, :],
                            wqv_d[4 * h + a:4 * h + b].rearrange("e p x -> p e x"))
                        nc.sync.dma_start(
                            wk_[:, a:b, :],
                            wk_d[4 * h + a:4 * h + b].rearrange("e p x -> p e x"))
                    wqvt.append(wq_)
                    wkt.append(wk_)

                def load_x(q, h, eng=None, split=False):
                    xe = xt_p.tile([P, 4, 512], F16, tag=f"xt{q}h{h}",
                                   name=f"xt{q}h{h}")
                    halves = [(0, 2), (2, 4)] if split else [(0, 4)]
                    for (a, b) in halves:
                        (eng or nc.sync).dma_start(
                            xe[:, a:b, :],
                            xtd[4 * h + a:4 * h + b, :, 512 * q:512 * (q + 1)]
                            .rearrange("e p l -> p e l"))
                    xth[q][h] = xe

                # first x chunk on the ACT HWDGE queue so it streams in
                # parallel with the weights on the SP queue (ACT is idle)
                load_x(0, 0, eng=nc.scalar, split=True)
                load_w(0, split=True)
                nc.sync.dma_start(cst[:], cst_d[:])
                nc.sync.dma_start(gb[:], gb_d[:])
                load_w(1)
                load_x(0, 1)
                for q in range(1, 4):
                    load_x(q, 0)
                    load_x(q, 1)

                def wqv(e):
                    return wqvt[e // 4][:, e % 4, :]

                def wk(e):
                    return wkt[e // 4][:, e % 4, :]

                def xt(e, q):
                    return xth[q][e // 4][:, e % 4, :]

                def emit_tr(qrow, krow, t):
                    # PE transpose to [d, l]; *g+b fused into the ACT store
                    for p_ in range(2):
                        trq = ps_tr.tile([P, P], F16, tag="tr")
                        nc.tensor.transpose(trq[:], qrow[:, 128 * p_:128 * (p_ + 1)], ident[:])
                        nc.scalar.activation(qt[p_][:, t * P:(t + 1) * P], trq[:],
                                             AF.Identity, bias=gb[:, 1:2], scale=gb[:, 0:1])
                        trk = ps_tr.tile([P, P], F16, tag="tr")
                        nc.tensor.transpose(trk[:], krow[:, 128 * p_:(p_ + 1) * 128], ident[:])
                        nc.scalar.activation(kt[p_][:, t * P:(t + 1) * P], trk[:],
                                             AF.Identity, bias=gb[:, 3:4], scale=gb[:, 2:3])

                prev_rows = None
                for t in range(T):
                    pq = ps_qv.tile([P, 512], F32, tag="pqv")
                    pk = ps_k.tile([P, 256], F32, tag="pk")
                    for e in range(E):
                        xchunk = xt(e, t // 4)[:, (t % 4) * P:(t % 4 + 1) * P]
                        nc.tensor.matmul(pq[:], xchunk, wqv(e),
                                         start=(e == 0), stop=(e == E - 1))
                        nc.tensor.matmul(pk[:], xchunk, wk(e),
                                         start=(e == 0), stop=(e == E - 1))
                    # transposes of the previous tile: PE stays a tile ahead of
                    # the DVE/ACT consumers below
                    if prev_rows is not None:
                        emit_tr(*prev_rows)
                    # LN stats: sum of squares per (l, head) for q and k
                    sq = stats_p.tile([P, 512], F32, tag="sq", bufs=2)
                    nc.scalar.activation(sq[:, 0:256], pq[:, 0:256], AF.Square)
                    nc.scalar.activation(sq[:, 256:512], pk[:], AF.Square)
                    ssq = stats_p.tile([P, 8], F32, tag="ssq")
                    nc.vector.tensor_reduce(
                        ssq[:], sq[:].rearrange("p (g d) -> p g d", d=D),
                        axis=mybir.AxisListType.X, op=ALU.add)
                    # rstd = 1/sqrt(ssq/64 + eps) = exp(-0.5*ln(ssq/64 + eps));
                    # Ln/Exp/Square/Identity/Copy share one ACT table set
                    lnv = stats_p.tile([P, 8], F32, tag="lnv")
                    nc.scalar.activation(lnv[:], ssq[:], AF.Ln,
                                         bias=epst[:], scale=1.0 / D)
                    rstd = stats_p.tile([P, 8], F32, tag="rstd")
                    nc.scalar.activation(rstd[:], lnv[:], AF.Exp, scale=-0.5)
                    # rows = psum * rstd (all 4 heads in one DVE op via a
                    # stride-0 broadcast of the per-head rstd)
                    qrow = rows_p.tile([P, 256], F16, tag="qrow")
                    krow = rows_p.tile([P, 256], F16, tag="krow")
                    nc.vector.tensor_tensor(
                        qrow[:].rearrange("p (h d) -> p h d", h=HPC),
                        pq[:, 0:256].rearrange("p (h d) -> p h d", h=HPC),
                        rstd[:, 0:4][:, :, None].broadcast_to((P, HPC, D)),
                        ALU.mult)
                    nc.vector.tensor_tensor(
                        krow[:].rearrange("p (h d) -> p h d", h=HPC),
                        pk[:].rearrange("p (h d) -> p h d", h=HPC),
                        rstd[:, 4:8][:, :, None].broadcast_to((P, HPC, D)),
                        ALU.mult)
                    # V -> vb[t] (strided into 65-wide head slots)
                    nc.vector.tensor_copy(
                        vb[t][:, :, 0:64],
                        pq[:, 256:512].rearrange("p (h d) -> p h d", h=HPC))
                    prev_rows = (qrow, krow, t)
                emit_tr(*prev_rows)

            # ============ Phase 2: attention + interleaved output proj ========
            with (
                tc.tile_pool(name="wo", bufs=1) as wo_p,
                tc.tile_pool(name="otb", bufs=1) as ot_p,
            ):
                wo = wo_p.tile([P, 2, EMB], F16, tag="wo")
                nc.sync.dma_start(wo[:], wo_d.rearrange("c p n -> p c n"))
                ot = [[ot_p.tile([P, 512], F16, tag=f"ot{p_}c{c}", name=f"ot{p_}c{c}")
                       for c in range(C)] for p_ in range(2)]

                with (
                    tc.tile_pool(name="pp", bufs=8) as p_pool,
                    tc.tile_pool(name="nrm", bufs=4) as nrm_p,
                    tc.tile_pool(name="ysb", bufs=4) as ysb_p,
                    tc.tile_pool(name="ps_s", bufs=3, space="PSUM") as ps_s,
                    tc.tile_pool(name="ps_o", bufs=2, space="PSUM") as ps_o,
                ):
                    def emit_yproj(c, t, tail=False):
                        # one row-tile of y = sum_p OT_p[:,t]^T @ WoT_p.
                        # Mid-attention tiles rotate through the O pool; tail
                        # tiles use the (by then idle) deeper S pool, split
                        # their copies across DVE/ACT, and stream each half
                        # out immediately on alternating HWDGE queues.
                        ysb = ysb_p.tile([P, EMB], F16, tag="ysb", name="ysb")
                        for eh in range(2):
                            yps = ps_o.tile([P, 512], F32, tag="o", name="yps")
                            for p_ in range(2):
                                nc.tensor.matmul(
                                    yps[:],
                                    ot[p_][c][:, (t % 4) * P:(t % 4 + 1) * P],
                                    wo[:, p_, 512 * eh:512 * (eh + 1)],
                                    start=(p_ == 0), stop=(p_ == 1))
                            dst = ysb[:, 512 * eh:512 * (eh + 1)]
                            if tail and eh == 1:
                                nc.scalar.activation(dst, yps[:], AF.Copy)
                            else:
                                nc.vector.tensor_copy(dst, yps[:])
                        nc.sync.dma_start(y_d[t * P:(t + 1) * P, :], ysb[:])

                    # Flat attention stream: the S/exp -> O software pipeline
                    # is carried ACROSS head boundaries so neither PE nor ACT
                    # drains at a transition, and yproj row-tiles of the
                    # previous chunk are emitted after each head to fill the
                    # PE's exp-wait bubbles (attention alone is ACT-limited).
                    prev = None

                    def flush_prev():
                        nonlocal prev
                        if prev is None:
                            return
                        pair, pt, opsum, h, jmax, post = prev
                        for (i, j, s) in pair:
                            nc.tensor.matmul(
                                opsum[0:65, s:512], vb[j][:, h, :],
                                pt[:, i, s:512],
                                start=(j == 0), stop=(j == jmax))
                        if post is not None:
                            post()
                        prev = None

                    # big chunks first: they fill the S/exp pipeline deeply on
                    # their own; the shallow chunks at the end get the
                    # previous chunk's yproj row-tiles as PE filler
                    chunk_order = [0, 2, 3, 1]
                    for ci, c in enumerate(chunk_order):
                        l0 = 512 * c
                        jmax = 4 * c + 3
                        for hi, (p_, hl) in enumerate(
                                [(0, 0), (0, 1), (1, 0), (1, 1)]):
                            h = 2 * p_ + hl
                            rows = slice(64 * hl, 64 * hl + 64)
                            opsum = ps_o.tile([P, 512], F32, tag="o", name="opsum")
                            for pp in range(2 * c + 2):
                                sps = ps_s.tile([P, 2, 512], F32, tag="s",
                                                name="sps")
                                pair = []
                                for i in range(2):
                                    j = 2 * pp + i
                                    s = max(0, 128 * j - l0)
                                    diag = 128 * j >= l0
                                    nc.tensor.matmul(
                                        sps[:, i, s:512],
                                        kt[p_][rows, j * P:(j + 1) * P],
                                        qt[p_][rows, l0 + s:l0 + 512],
                                        start=True, stop=not diag)
                                    if diag:
                                        # diag 128x128 causal mask (-1e30)
                                        nc.tensor.matmul(
                                            sps[:, i, s:s + 128],
                                            ident[:], maskf[:],
                                            start=False, stop=True)
                                    pair.append((i, j, s))
                                # one exp per j-pair; clipped-off psum cols are
                                # stale-but-finite and never read.  The last
                                # pair's 384-col gap is worth a second
                                # instruction; smaller gaps are not.
                                s0, s1 = pair[0][2], pair[1][2]
                                pt = p_pool.tile([P, 2, 512], BF16, tag="p",
                                                 name="pt")
                                if s1 >= 384:
                                    nc.scalar.activation(
                                        pt[:, 0, s0:512], sps[:, 0, s0:512],
                                        AF.Exp)
                                    nc.scalar.activation(
                                        pt[:, 1, s1:512], sps[:, 1, s1:512],
                                        AF.Exp)
                                else:
                                    nc.scalar.activation(
                                        pt[:].rearrange("p t x -> p (t x)")[:, s0:1024],
                                        sps[:].rearrange("p t x -> p (t x)")[:, s0:1024],
                                        AF.Exp)
                                flush_prev()
                                post = None
                                if pp == 2 * c + 1:
                                    def post(p_=p_, hl=hl, c=c, opsum=opsum):
                                        # normalize rows by the sums row (64)
                                        rec = nrm_p.tile([1, 512], F32,
                                                         tag="rec", name="rec")
                                        nc.vector.reciprocal(
                                            rec[:], opsum[64:65, :])
                                        recb = nrm_p.tile([64, 512], F32,
                                                          tag="recb", name="recb")
                                        nc.gpsimd.partition_broadcast(
                                            recb[:], rec[:])
                                        nc.vector.tensor_tensor(
                                            ot[p_][c][64 * hl:64 * hl + 64, :],
                                            opsum[0:64, :], recb[:], ALU.mult)
                                prev = (pair, pt, opsum, h, jmax, post)
                            # yproj of the previously-completed chunk, one
                            # row-tile per head
                            if ci > 0:
                                pc = chunk_order[ci - 1]
                                emit_yproj(pc, 4 * pc + hi)
                    flush_prev()
                    # last chunk's yproj drains at the tail: split the
                    # PSUM->SBUF copies across DVE and ACT
                    lc = chunk_order[-1]
                    for t in range(4 * lc, 4 * lc + 4):
                        emit_yproj(lc, t, tail=True)

    nc.compile()
    return nc


_NC = None


def _get_nc():
    global _NC
    if _NC is None:
        _NC = build_nc()
    return _NC


def _center(w):
    # fold LayerNorm mean-subtraction into the projection weights (per head)
    w3 = w.astype(np.float64).reshape(-1, D, EMB)
    w3 = w3 - w3.mean(axis=1, keepdims=True)
    return w3.reshape(-1, EMB)


def make_in_maps(x, Wq, Wk, Wv, gq, bq, gk, bk, Wo):
    BF = ml_dtypes.bfloat16
    F16 = np.float16
    x = np.asarray(x, np.float32)
    Wq = np.asarray(Wq, np.float32)
    Wk = np.asarray(Wk, np.float32)
    Wv = np.asarray(Wv, np.float32)
    Wo = np.asarray(Wo, np.float32)
    gq = np.asarray(gq, np.float32)
    bq = np.asarray(bq, np.float32)
    gk = np.asarray(gk, np.float32)
    bk = np.asarray(bk, np.float32)

    ident = np.eye(P, dtype=F16)
    # additive causal mask for the diagonal 128x128 block of ST[m, l_local]:
    # invalid where l < m
    maskf = np.where(np.arange(P)[None, :] < np.arange(P)[:, None], NEG, 0.0
                     ).astype(F16)
    cst = np.stack([ident, maskf], axis=1)  # [P, 2, P]
    gb = np.stack([np.tile(gq, 2), np.tile(bq, 2), np.tile(gk, 2), np.tile(bk, 2)],
                  axis=1).astype(np.float32)  # [128, 4]

    in_maps = []
    for cix in range(NCORES):
        n, g = divmod(cix, HPC)
        rows = slice(256 * g, 256 * (g + 1))
        xT = np.ascontiguousarray(x[n].T).astype(F16)
        wqT = np.ascontiguousarray(_center(Wq[rows]).T).reshape(E, P, 256)
        wvT = np.ascontiguousarray(Wv[rows].astype(np.float64).T).reshape(E, P, 256)
        wqv = np.concatenate([wqT, wvT], axis=2).astype(F16)
        wkT = np.ascontiguousarray(_center(Wk[rows]).T).reshape(E, P, 256).astype(F16)
        woT = np.ascontiguousarray(Wo[:, rows].T).reshape(2, P, EMB).astype(F16)
        in_maps.append({
            "xT": xT, "wqv": np.ascontiguousarray(wqv), "wk": wkT, "wo": woT,
            "cst": cst, "gb": gb,
        })
    return in_maps


def kernel(x, mask, Wq, Wk, Wv, gq, bq, gk, bk, Wo, bo):
    nc = _get_nc()
    in_maps = make_in_maps(x, Wq, Wk, Wv, gq, bq, gk, bk, Wo)
    res = run_bass_kernel_spmd(nc, in_maps, list(range(NCORES)))
    bo = np.asarray(bo, np.float32)
    y = np.zeros((2, L, EMB), np.float32)
    for n in range(2):
        acc = np.zeros((L, EMB), np.float32)
        for g in range(HPC):
            acc += np.asarray(res.results[HPC * n + g]["y"], dtype=np.float32)
        y[n] = acc + bo[None, :]
    return y


# revision 26
# speedup vs baseline: 1.0069x; 1.0069x over previous
"""Trainium2 Bass kernel for CustomMultiHeadSelfAttention (fused q/k LayerNorm).

Reference computation (per batch n):
    q = x @ Wq.T ; k = x @ Wk.T ; v = x @ Wv.T          (split into 16 heads of 64)
    q = LN_head(q) * gq + bq ; k = LN_head(k) * gk + bk  (LayerNorm over head_dim)
    out = causal_softmax(q @ k.T) @ v                    (per head)
    y = concat_heads(out) @ Wo.T + bo

Sharding: 8 cores = 2 batches x 4 head-groups (4 heads each).  Each core
computes its heads' attention and a partial y = out_heads @ Wo[:, cols].T;
the host sums the 4 partials per batch (fp32) and adds bo.

Device-side dataflow per core (all matmul operands bf16, PSUM accum fp32):
  - QKV projection from streamed xT chunks; LayerNorm mean is folded into
    the Q/K weights on the host (centered weights => mean(q)=0), so LN
    reduces to q * rsqrt(mean(q^2)+eps).
  - Engine split chosen so ACT only ever runs functions from ONE table set
    (natural_log_exp_and_others: Exp/Ln/Square/Identity/Copy -> a single
    table load): squares for LN stats on ACT, grouped sum on DVE,
    rstd = exp(-0.5*ln(ms+eps)) on ACT, q/k scaling merged per-tile on DVE
    (stride-0 broadcast of per-head rstd), PE transpose to [d, l] (bf16,
    1 cyc/row), *g+b fused into the ACT Identity PSUM->SBUF store.
    Transposes are software-pipelined one tile behind the projection
    matmuls so the PE never waits on the DVE/ACT consumers.
  - Attention is l-chunk-major: for each 512-wide query chunk c, S^T for
    m-chunks j=0..4c+3 is computed in j-pairs into a [P, 2, 512] PSUM
    tile, exp'd in one ACT pass (fp32->bf16, no max subtraction: LN
    bounds |score| <= 64), and accumulated V-stationary into a single
    per-head O PSUM bank: OT_aug[65, 512] += V_aug[m,65]^T @ P[m, 512]
    (V_aug has a ones column -> row 64 = softmax denominators).  The
    causal diagonal 128x128 blocks get a -1e30 additive mask via an
    identity matmul.  O rows are scaled by 1/sums (DVE reciprocal +
    GPSIMD partition-broadcast + DVE multiply) into bf16 OT_heads.
  - One O bank per head (vs 4 in a j-major sweep) leaves 2 PSUM banks for
    the output projection, which is interleaved per chunk right after the
    chunk's last head is normalized: y[t] = sum_p OT_p[:,t]^T @ WoT_p,
    PSUM->SBUF bf16 copies alternating DVE/ACT, streamed to DRAM as bf16.
"""

import ml_dtypes
import numpy as np

import concourse.bass as bass
import concourse.tile as tile
from concourse import bacc, mybir
from concourse.bass_utils import run_bass_kernel_spmd

F32 = mybir.dt.float32
BF16 = mybir.dt.bfloat16
F16 = mybir.dt.float16

P = 128
EMB = 1024
L = 2048
D = 64
HPC = 4           # heads per core
NCORES = 8
EPS = 1e-5
NEG = -60000.0  # f16-finite; scores are bounded by 64, exp(s+NEG) == 0
T = L // P        # 16 l-tiles
E = EMB // P      # 8 emb chunks
C = 4             # 512-wide query chunks
AF = mybir.ActivationFunctionType
ALU = mybir.AluOpType


def build_nc():
    nc = bacc.Bacc("TRN2", target_bir_lowering=False, debug=False, num_devices=NCORES)

    xT_d = nc.dram_tensor("xT", [EMB, L], F16, kind="ExternalInput")
    wqv_d = nc.dram_tensor("wqv", [E, P, 512], F16, kind="ExternalInput")
    wk_d = nc.dram_tensor("wk", [E, P, 256], F16, kind="ExternalInput")
    wo_d = nc.dram_tensor("wo", [2, P, EMB], F16, kind="ExternalInput")
    cst_d = nc.dram_tensor("cst", [P, 2, P], F16, kind="ExternalInput")  # ident|maskf
    gb_d = nc.dram_tensor("gb", [P, 4], F32, kind="ExternalInput")  # gq2 bq2 gk2 bk2
    y_d = nc.dram_tensor("y", [L, EMB], F16, kind="ExternalOutput")

    with tile.TileContext(nc) as tc:
        # ---- persistent pools (bottom of the SBUF stack) ----
        with (
            tc.tile_pool(name="const", bufs=1) as const_p,
            tc.tile_pool(name="vbuf", bufs=1) as vbuf_p,
            tc.tile_pool(name="qtkt", bufs=1) as qtkt_p,
        ):
            cst = const_p.tile([P, 2, P], F16, tag="cst")
            ident = cst[:, 0, :]
            maskf = cst[:, 1, :]
            gb = const_p.tile([P, 4], F32, tag="gb")
            epst = const_p.tile([P, 1], F32, tag="epst")
            # all ACT functions used below (Square/Ln/Exp/Identity/Copy) live
            # in table set 6 (natural_log_exp_and_others); pre-placing the
            # load keeps the fixpoint pass from greedy-thrashing between sets
            nc.scalar.add_instruction(mybir.InstLoadActFuncSet(
                name=nc.get_next_instruction_name(), act_func_set_id=6,
                ins=[], outs=[]))
            nc.vector.memset(epst[:], EPS)

            # V with a ones column per head: vb[t][:, h, 0:65]
            vb = []
            for t in range(T):
                v_ = vbuf_p.tile([P, HPC, 65], BF16, tag=f"vb{t}", name=f"vb{t}")
                nc.gpsimd.memset(v_[:], 1.0)  # ones col survives V copy
                vb.append(v_)

            # QT/KT: head pair p occupies rows [0:64]=head 2p, [64:128]=head 2p+1
            qt = [qtkt_p.tile([P, L], F16, tag=f"qt{p_}", name=f"qt{p_}") for p_ in range(2)]
            kt = [qtkt_p.tile([P, L], F16, tag=f"kt{p_}", name=f"kt{p_}") for p_ in range(2)]

            # ================= Phase 1: projections + LN + transpose ==========
            with (
                tc.tile_pool(name="xt", bufs=1) as xt_p,
                tc.tile_pool(name="wts", bufs=1) as wts_p,
                tc.tile_pool(name="rows", bufs=3) as rows_p,
                tc.tile_pool(name="stats", bufs=3) as stats_p,
                tc.tile_pool(name="ps_qv", bufs=3, space="PSUM") as ps_qv,
                tc.tile_pool(name="ps_k", bufs=3, space="PSUM") as ps_k,
                tc.tile_pool(name="ps_tr", bufs=2, space="PSUM") as ps_tr,
            ):
                # batched input streaming: every dma_start costs ~625ns of
                # HWDGE descriptor generation, so issue few large transfers
                # ordered so tile 0's first matmuls unblock earliest
                xtd = xT_d.rearrange("(e p) l -> e p l", p=P)
                wqvt, wkt = [], []
                xth = [[None, None] for _ in range(4)]  # [quarter][e-half]

                def load_w(h, split=False):
                    wq_ = wts_p.tile([P, 4, 512], F16, tag=f"wqv{h}", name=f"wqv{h}")
                    wk_ = wts_p.tile([P, 4, 256], F16, tag=f"wk{h}", name=f"wk{h}")
                    halves = [(0, 2), (2, 4)] if split else [(0, 4)]
                    for (a, b) in halves:
                        nc.sync.dma_start(
                            wq_[:, a:b, :],
                            wqv_d[4 * h + a:4 * h + b].rearrange("e p x -> p e x"))
                        nc.sync.dma_start(
                            wk_[:, a:b, :],
                            wk_d[4 * h + a:4 * h + b].rearrange("e p x -> p e x"))
                    wqvt.append(wq_)
                    wkt.append(wk_)

                def load_x(q, h, eng=None, split=False):
                    xe = xt_p.tile([P, 4, 512], F16, tag=f"xt{q}h{h}",
                                   name=f"xt{q}h{h}")
                    halves = [(0, 2), (2, 4)] if split else [(0, 4)]
                    for (a, b) in halves:
                        (eng or nc.sync).dma_start(
                            xe[:, a:b, :],
                            xtd[4 * h + a:4 * h + b, :, 512 * q:512 * (q + 1)]
                            .rearrange("e p l -> p e l"))
                    xth[q][h] = xe

                # first x chunk on the ACT HWDGE queue so it streams in
                # parallel with the weights on the SP queue (ACT is idle)
                load_x(0, 0, eng=nc.scalar)
                load_w(0, split=True)
                nc.sync.dma_start(cst[:], cst_d[:])
                nc.sync.dma_start(gb[:], gb_d[:])
                load_w(1)
                load_x(0, 1)
                for q in range(1, 4):
                    load_x(q, 0)
                    load_x(q, 1)

                def wqv(e):
                    return wqvt[e // 4][:, e % 4, :]

                def wk(e):
                    return wkt[e // 4][:, e % 4, :]

                def xt(e, q):
                    return xth[q][e // 4][:, e % 4, :]

                def emit_tr(qrow, krow, t):
                    # PE transpose to [d, l]; *g+b fused into the ACT store
                    for p_ in range(2):
                        trq = ps_tr.tile([P, P], F16, tag="tr")
                        nc.tensor.transpose(trq[:], qrow[:, 128 * p_:128 * (p_ + 1)], ident[:])
                        nc.scalar.activation(qt[p_][:, t * P:(t + 1) * P], trq[:],
                                             AF.Identity, bias=gb[:, 1:2], scale=gb[:, 0:1])
                        trk = ps_tr.tile([P, P], F16, tag="tr")
                        nc.tensor.transpose(trk[:], krow[:, 128 * p_:(p_ + 1) * 128], ident[:])
                        nc.scalar.activation(kt[p_][:, t * P:(t + 1) * P], trk[:],
                                             AF.Identity, bias=gb[:, 3:4], scale=gb[:, 2:3])

                prev_rows = None
                for t in range(T):
                    pq = ps_qv.tile([P, 512], F32, tag="pqv")
                    pk = ps_k.tile([P, 256], F32, tag="pk")
                    for e in range(E):
                        xchunk = xt(e, t // 4)[:, (t % 4) * P:(t % 4 + 1) * P]
                        nc.tensor.matmul(pq[:], xchunk, wqv(e),
                                         start=(e == 0), stop=(e == E - 1))
                        nc.tensor.matmul(pk[:], xchunk, wk(e),
                                         start=(e == 0), stop=(e == E - 1))
                    # transposes of the previous tile: PE stays a tile ahead of
                    # the DVE/ACT consumers below
                    if prev_rows is not None:
                        emit_tr(*prev_rows)
                    # LN stats: sum of squares per (l, head) for q and k
                    sq = stats_p.tile([P, 512], F32, tag="sq", bufs=2)
                    nc.scalar.activation(sq[:, 0:256], pq[:, 0:256], AF.Square)
                    nc.scalar.activation(sq[:, 256:512], pk[:], AF.Square)
                    ssq = stats_p.tile([P, 8], F32, tag="ssq")
                    nc.vector.tensor_reduce(
                        ssq[:], sq[:].rearrange("p (g d) -> p g d", d=D),
                        axis=mybir.AxisListType.X, op=ALU.add)
                    # rstd = 1/sqrt(ssq/64 + eps) = exp(-0.5*ln(ssq/64 + eps));
                    # Ln/Exp/Square/Identity/Copy share one ACT table set
                    lnv = stats_p.tile([P, 8], F32, tag="lnv")
                    nc.scalar.activation(lnv[:], ssq[:], AF.Ln,
                                         bias=epst[:], scale=1.0 / D)
                    rstd = stats_p.tile([P, 8], F32, tag="rstd")
                    nc.scalar.activation(rstd[:], lnv[:], AF.Exp, scale=-0.5)
                    # rows = psum * rstd (all 4 heads in one DVE op via a
                    # stride-0 broadcast of the per-head rstd)
                    qrow = rows_p.tile([P, 256], F16, tag="qrow")
                    krow = rows_p.tile([P, 256], F16, tag="krow")
                    nc.vector.tensor_tensor(
                        qrow[:].rearrange("p (h d) -> p h d", h=HPC),
                        pq[:, 0:256].rearrange("p (h d) -> p h d", h=HPC),
                        rstd[:, 0:4][:, :, None].broadcast_to((P, HPC, D)),
                        ALU.mult)
                    nc.vector.tensor_tensor(
                        krow[:].rearrange("p (h d) -> p h d", h=HPC),
                        pk[:].rearrange("p (h d) -> p h d", h=HPC),
                        rstd[:, 4:8][:, :, None].broadcast_to((P, HPC, D)),
                        ALU.mult)
                    # V -> vb[t] (strided into 65-wide head slots)
                    nc.vector.tensor_copy(
                        vb[t][:, :, 0:64],
                        pq[:, 256:512].rearrange("p (h d) -> p h d", h=HPC))
                    prev_rows = (qrow, krow, t)
                emit_tr(*prev_rows)

            # ============ Phase 2: attention + interleaved output proj ========
            with (
                tc.tile_pool(name="wo", bufs=1) as wo_p,
                tc.tile_pool(name="otb", bufs=1) as ot_p,
            ):
                wo = wo_p.tile([P, 2, EMB], F16, tag="wo")
                nc.sync.dma_start(wo[:], wo_d.rearrange("c p n -> p c n"))
                ot = [[ot_p.tile([P, 512], F16, tag=f"ot{p_}c{c}", name=f"ot{p_}c{c}")
                       for c in range(C)] for p_ in range(2)]

                with (
                    tc.tile_pool(name="pp", bufs=8) as p_pool,
                    tc.tile_pool(name="nrm", bufs=4) as nrm_p,
                    tc.tile_pool(name="ysb", bufs=4) as ysb_p,
                    tc.tile_pool(name="ps_s", bufs=3, space="PSUM") as ps_s,
                    tc.tile_pool(name="ps_o", bufs=2, space="PSUM") as ps_o,
                ):
                    def emit_yproj(c, t, tail=False):
                        # one row-tile of y = sum_p OT_p[:,t]^T @ WoT_p.
                        # Mid-attention tiles rotate through the O pool; tail
                        # tiles use the (by then idle) deeper S pool, split
                        # their copies across DVE/ACT, and stream each half
                        # out immediately on alternating HWDGE queues.
                        ysb = ysb_p.tile([P, EMB], F16, tag="ysb", name="ysb")
                        for eh in range(2):
                            yps = ps_o.tile([P, 512], F32, tag="o", name="yps")
                            for p_ in range(2):
                                nc.tensor.matmul(
                                    yps[:],
                                    ot[p_][c][:, (t % 4) * P:(t % 4 + 1) * P],
                                    wo[:, p_, 512 * eh:512 * (eh + 1)],
                                    start=(p_ == 0), stop=(p_ == 1))
                            dst = ysb[:, 512 * eh:512 * (eh + 1)]
                            if tail and eh == 1:
                                nc.scalar.activation(dst, yps[:], AF.Copy)
                            else:
                                nc.vector.tensor_copy(dst, yps[:])
                        nc.sync.dma_start(y_d[t * P:(t + 1) * P, :], ysb[:])

                    # Flat attention stream: the S/exp -> O software pipeline
                    # is carried ACROSS head boundaries so neither PE nor ACT
                    # drains at a transition, and yproj row-tiles of the
                    # previous chunk are emitted after each head to fill the
                    # PE's exp-wait bubbles (attention alone is ACT-limited).
                    prev = None

                    def flush_prev():
                        nonlocal prev
                        if prev is None:
                            return
                        pair, pt, opsum, h, jmax, post = prev
                        for (i, j, s) in pair:
                            nc.tensor.matmul(
                                opsum[0:65, s:512], vb[j][:, h, :],
                                pt[:, i, s:512],
                                start=(j == 0), stop=(j == jmax))
                        if post is not None:
                            post()
                        prev = None

                    # big chunks first: they fill the S/exp pipeline deeply on
                    # their own; the shallow chunks at the end get the
                    # previous chunk's yproj row-tiles as PE filler
                    chunk_order = [0, 2, 3, 1]
                    for ci, c in enumerate(chunk_order):
                        l0 = 512 * c
                        jmax = 4 * c + 3
                        for hi, (p_, hl) in enumerate(
                                [(0, 0), (0, 1), (1, 0), (1, 1)]):
                            h = 2 * p_ + hl
                            rows = slice(64 * hl, 64 * hl + 64)
                            opsum = ps_o.tile([P, 512], F32, tag="o", name="opsum")
                            for pp in range(2 * c + 2):
                                sps = ps_s.tile([P, 2, 512], F32, tag="s",
                                                name="sps")
                                pair = []
                                for i in range(2):
                                    j = 2 * pp + i
                                    s = max(0, 128 * j - l0)
                                    diag = 128 * j >= l0
                                    nc.tensor.matmul(
                                        sps[:, i, s:512],
                                        kt[p_][rows, j * P:(j + 1) * P],
                                        qt[p_][rows, l0 + s:l0 + 512],
                                        start=True, stop=not diag)
                                    if diag:
                                        # diag 128x128 causal mask (-1e30)
                                        nc.tensor.matmul(
                                            sps[:, i, s:s + 128],
                                            ident[:], maskf[:],
                                            start=False, stop=True)
                                    pair.append((i, j, s))
                                # one exp per j-pair; clipped-off psum cols are
                                # stale-but-finite and never read.  The last
                                # pair's 384-col gap is worth a second
                                # instruction; smaller gaps are not.
                                s0, s1 = pair[0][2], pair[1][2]
                                pt = p_pool.tile([P, 2, 512], BF16, tag="p",
                                                 name="pt")
                                if s1 >= 384:
                                    nc.scalar.activation(
                                        pt[:, 0, s0:512], sps[:, 0, s0:512],
                                        AF.Exp)
                                    nc.scalar.activation(
                                        pt[:, 1, s1:512], sps[:, 1, s1:512],
                                        AF.Exp)
                                else:
                                    nc.scalar.activation(
                                        pt[:].rearrange("p t x -> p (t x)")[:, s0:1024],
                                        sps[:].rearrange("p t x -> p (t x)")[:, s0:1024],
                                        AF.Exp)
                                flush_prev()
                                post = None
                                if pp == 2 * c + 1:
                                    def post(p_=p_, hl=hl, c=c, opsum=opsum):
                                        # normalize rows by the sums row (64)
                                        rec = nrm_p.tile([1, 512], F32,
                                                         tag="rec", name="rec")
                                        nc.vector.reciprocal(
                                            rec[:], opsum[64:65, :])
                                        recb = nrm_p.tile([64, 512], F32,
                                                          tag="recb", name="recb")
                                        nc.gpsimd.partition_broadcast(
                                            recb[:], rec[:])
                                        nc.vector.tensor_tensor(
                                            ot[p_][c][64 * hl:64 * hl + 64, :],
                                            opsum[0:64, :], recb[:], ALU.mult)
                                prev = (pair, pt, opsum, h, jmax, post)
                            # yproj of the previously-completed chunk, one
                            # row-tile per head
                            if ci > 0:
                                pc = chunk_order[ci - 1]
                                emit_yproj(pc, 4 * pc + hi)
                    flush_prev()
                    # last chunk's yproj drains at the tail: split the
                    # PSUM->SBUF copies across DVE and ACT
                    lc = chunk_order[-1]
                    for t in range(4 * lc, 4 * lc + 4):
                        emit_yproj(lc, t, tail=True)

    nc.compile()
    return nc


_NC = None


def _get_nc():
    global _NC
    if _NC is None:
        _NC = build_nc()
    return _NC


def _center(w):
    # fold LayerNorm mean-subtraction into the projection weights (per head)
    w3 = w.astype(np.float64).reshape(-1, D, EMB)
    w3 = w3 - w3.mean(axis=1, keepdims=True)
    return w3.reshape(-1, EMB)


def make_in_maps(x, Wq, Wk, Wv, gq, bq, gk, bk, Wo):
    BF = ml_dtypes.bfloat16
    F16 = np.float16
    x = np.asarray(x, np.float32)
    Wq = np.asarray(Wq, np.float32)
    Wk = np.asarray(Wk, np.float32)
    Wv = np.asarray(Wv, np.float32)
    Wo = np.asarray(Wo, np.float32)
    gq = np.asarray(gq, np.float32)
    bq = np.asarray(bq, np.float32)
    gk = np.asarray(gk, np.float32)
    bk = np.asarray(bk, np.float32)

    ident = np.eye(P, dtype=F16)
    # additive causal mask for the diagonal 128x128 block of ST[m, l_local]:
    # invalid where l < m
    maskf = np.where(np.arange(P)[None, :] < np.arange(P)[:, None], NEG, 0.0
                     ).astype(F16)
    cst = np.stack([ident, maskf], axis=1)  # [P, 2, P]
    gb = np.stack([np.tile(gq, 2), np.tile(bq, 2), np.tile(gk, 2), np.tile(bk, 2)],
                  axis=1).astype(np.float32)  # [128, 4]

    in_maps = []
    for cix in range(NCORES):
        n, g = divmod(cix, HPC)
        rows = slice(256 * g, 256 * (g + 1))
        xT = np.ascontiguousarray(x[n].T).astype(F16)
        wqT = np.ascontiguousarray(_center(Wq[rows]).T).reshape(E, P, 256)
        wvT = np.ascontiguousarray(Wv[rows].astype(np.float64).T).reshape(E, P, 256)
        wqv = np.concatenate([wqT, wvT], axis=2).astype(F16)
        wkT = np.ascontiguousarray(_center(Wk[rows]).T).reshape(E, P, 256).astype(F16)
        woT = np.ascontiguousarray(Wo[:, rows].T).reshape(2, P, EMB).astype(F16)
        in_maps.append({
            "xT": xT, "wqv": np.ascontiguousarray(wqv), "wk": wkT, "wo": woT,
            "cst": cst, "gb": gb,
        })
    return in_maps


def kernel(x, mask, Wq, Wk, Wv, gq, bq, gk, bk, Wo, bo):
    nc = _get_nc()
    in_maps = make_in_maps(x, Wq, Wk, Wv, gq, bq, gk, bk, Wo)
    res = run_bass_kernel_spmd(nc, in_maps, list(range(NCORES)))
    bo = np.asarray(bo, np.float32)
    y = np.zeros((2, L, EMB), np.float32)
    for n in range(2):
        acc = np.zeros((L, EMB), np.float32)
        for g in range(HPC):
            acc += np.asarray(res.results[HPC * n + g]["y"], dtype=np.float32)
        y[n] = acc + bo[None, :]
    return y
